# revision 24
# baseline (speedup 1.0000x reference)
"""LoRALinear Trainium2 kernel.

out = x @ W^T + bias + 2.0 * ((x @ A^T) @ B^T)

Strategy (v15):
  - 2x4 sharding over 8 NeuronCores: M split 2-way, out_features 4-way.
    Per core: x-shard [4096 tok, 4096 k], W-shard [1024 out, 4096 k].
  - Host ships k-major pre-tiled f16 layouts (layout + dtype cast only):
    x as [16 pair, 128 p, 32 kt, 256 m] f16 (contiguous 2MiB per pair
    fetch); W as [32 kt, 128 p, 1024 o] f16, DMA'd per k-tile straight
    into its wf tile; A and B^T zero-padded to 128 partitions so their
    DMAs run at line rate (16-partition DMAs only use 2 of 16 ports and
    landed ~14us, stalling the whole BA chain in v6/v7).
  - LoRA fold: rank-16 BA matmul lands in PSUM, then ONE in-place DVE
    add per 512-col half: wf[kt] += (2*B@A)^T[kt]. W_eff production is
    DVE-paced at ~1.2us/kt and overlaps the W/x DMA window (~40us).
  - Mixed precision: k-tiles 0..7 as fp8e4 DoubleRow pairs (4 DR passes
    of FD=512 per group, each covering 2 k-tiles), k-tiles 8..31 f16
    (24 passes of FD=512). fp8 operands scaled W*8 / x*(1/8) (exact
    pow2). ACT-engine f16->f8 conversions are bit-exact RNE
    (probe-verified). Measured rel_max vs cpu reference: 1.87e-2
    (gate 2e-2).
  - Group order: DR kp0 (start=True, full-bank FD=512) .. kp3, then f16
    kt8..kt30, kt31 carries stop; DVE adds bias at eviction.
  - Head: m-tiles 0,1 (x pair 0) x 2 oc absorb the W stream k-outer,
    lagging the fold chain by two k-tiles so absorption passes neither
    join the fold critical path in the Tensor FIFO nor wait on a
    just-landed fold. x pair 1 is fetched late (kt26) to keep the W
    stream at full DMA rate. Steady phase is k-inner per m-tile; final
    m-tile stores per-oc halves.
"""

import numpy as np

IN_F = 4096
OUT_F = 4096
R = 16
SCALING = 2.0
M = 4 * 2048  # 8192 tokens

N_CORES = 8
M_SPLIT = 2
O_SPLIT = 4
M_SH = M // M_SPLIT      # 4096 rows per core
O_SH = OUT_F // O_SPLIT  # 1024 out-features per core
K = IN_F
KT = K // 128            # 32 k-tiles
N_KP = 4                 # fp8 k-pairs = k-tiles 0..7
KT16_0 = 2 * N_KP        # first f16 k-tile (8)
N_MT = M_SH // 128       # 32 m-tiles
N_PAIR = N_MT // 2       # 16 m-tile pairs (x fetched per pair)
WS8 = 8.0                # fp8 scaling: W*8, x/8

_NC_CACHE = {}
LAST_RESULT = None


def _build():
    import concourse.mybir as mybir
    import concourse.tile as tile
    from concourse import bacc

    f32, f16, f8 = mybir.dt.float32, mybir.dt.float16, mybir.dt.float8e4
    DR = mybir.MatmulPerfMode.DoubleRow
    COPY = mybir.ActivationFunctionType.Copy

    nc = bacc.Bacc(
        "TRN2", target_bir_lowering=False, debug=False, num_devices=N_CORES
    )
    xtl_d = nc.dram_tensor("xtl", [N_PAIR, 128, KT - KT16_0, 256], f16,
                           kind="ExternalInput")
    xq_d = nc.dram_tensor("xq", [N_PAIR, 128, N_KP, 2, 256], f8,
                          kind="ExternalInput")
    wtl_d = nc.dram_tensor("wtl", [KT, 128, O_SH], f16, kind="ExternalInput")
    a_d = nc.dram_tensor("a", [128, K], f16, kind="ExternalInput")
    bt_d = nc.dram_tensor("bt", [128, O_SH], f16, kind="ExternalInput")
    bias_d = nc.dram_tensor("bias", [O_SH], f32, kind="ExternalInput")
    out_d = nc.dram_tensor("out", [M_SH, O_SH], f32, kind="ExternalOutput")

    with tile.TileContext(nc) as tc:
        with (
            tc.tile_pool(name="const", bufs=1) as const,
            tc.tile_pool(name="wfp", bufs=1) as wfp,
            tc.tile_pool(name="wp8p", bufs=1) as wp8p,
            tc.tile_pool(name="xfp", bufs=3) as xfp,
            tc.tile_pool(name="xpp", bufs=3) as xpp,
            tc.tile_pool(name="outp", bufs=3) as outp,
        ):
            # ---- constants ----
            junk = const.tile([128, 512], f16)
            nc.vector.memset(junk[:], 0.0)
            # A / B^T are zero-padded to 128 partitions for line-rate DMA
            a_sb = const.tile([128, K], f16)
            nc.sync.dma_start(a_sb[:], a_d[:])
            bt2 = const.tile([128, O_SH], f16)  # host ships 2*B^T (exact
            nc.sync.dma_start(bt2[:], bt_d[:])   # pow2 scale, bit-lossless)
            bias_bc = const.tile([128, O_SH], f32)
            nc.gpsimd.dma_start(bias_bc[:], bias_d[:].partition_broadcast(128))

            wf = {kt: wfp.tile([128, O_SH], f16, name=f"wf{kt}")
                  for kt in range(KT)}
            wp8 = [wp8p.tile([128, 2, O_SH], f8, name=f"wp8_{kp}")
                   for kp in range(N_KP)]

            def w_dma(kt):
                nc.sync.dma_start(wf[kt][:], wtl_d[kt, :, :])

            def ba_fold(kt, pba_pool):
                """BA matmul; fold in-place: wf[kt] += (2*B@A)^T[kt]."""
                ks = slice(kt * 128, (kt + 1) * 128)
                for h in range(2):
                    hs = slice(h * 512, (h + 1) * 512)
                    pba = pba_pool.tile([128, 512], f32, name=f"pba{h}")
                    nc.tensor.matmul(
                        pba[:], a_sb[0:R, ks], bt2[0:R, hs],
                        start=True, stop=True,
                    )
                    nc.vector.tensor_add(wf[kt][:, hs], pba[:], wf[kt][:, hs])

            xfs = [None] * N_PAIR  # f16 x tiles [128, KT, 256] per m-pair
            xps = [None] * N_PAIR  # fp8 paired x tiles [128, N_KP, 2, 256]

            def fetch_x(pr):
                # fp8 x ships pre-quantized from host (bit-identical to the
                # ACT path); the tiny xq lands first so DR passes never wait
                # on the 1.5MiB f16 fetch
                xp = xpp.tile([128, N_KP, 2, 256], f8, name="xp")
                nc.sync.dma_start(xp[:], xq_d[pr])
                xf = xfp.tile([128, KT - KT16_0, 256], f16, name="xf")
                nc.sync.dma_start(xf[:], xtl_d[pr, :, :, :])
                xfs[pr], xps[pr] = xf, xp

            def xslice(mt, kt):
                pr, half = divmod(mt, 2)
                return xfs[pr][:, kt - KT16_0, half * 128 : half * 128 + 128]

            def xslice8(mt, kp):
                pr, half = divmod(mt, 2)
                return xps[pr][:, kp, :, half * 128 : half * 128 + 128]

            def dr_pass(mt, oc, pm, kp, start=False):
                ocs = slice(oc * 512, (oc + 1) * 512)
                nc.tensor.matmul(
                    pm[:], xslice8(mt, kp), wp8[kp][:, :, ocs],
                    start=start, stop=False, perf_mode=DR,
                )

            def f16_pass(mt, oc, pm, kt, start=False):
                nc.tensor.matmul(
                    pm[:], xslice(mt, kt), wf[kt][:, oc * 512 : (oc + 1) * 512],
                    start=start, stop=False,
                )

            def close_group(mt, oc, pm, osb):
                nc.tensor.matmul(
                    pm[:], xslice(mt, KT - 1),
                    wf[KT - 1][:, oc * 512 : (oc + 1) * 512],
                    start=False, stop=True,
                )
                hs = slice(oc * 512, (oc + 1) * 512)
                nc.vector.tensor_add(osb[:, hs], pm[:], bias_bc[:, hs])

            HEAD_MTS = (0, 1)

            with tc.tile_pool(name="bap", bufs=2, space="PSUM") as bap, \
                 tc.tile_pool(name="hps", bufs=1, space="PSUM") as hps:
                # warmup burst flips the PE clock gate early and bridges the
                # gap until the BA stream's inputs land (~9us)
                pwarm = bap.tile([128, 512], f32, name="pba0")
                for _ in range(24):
                    nc.tensor.matmul(
                        pwarm[:], junk[:, 0:128], junk[:], start=True, stop=True
                    )

                hpm = {
                    (mt, oc): hps.tile([128, 512], f32, name=f"h{mt}_{oc}")
                    for mt in HEAD_MTS for oc in range(2)
                }

                # fp8 k-range first: W + BA folds for kt 0..7, quantize, then
                # open the head groups with a dense DR burst (re-arms HAM)
                for kt in range(KT16_0):
                    w_dma(kt)
                    ba_fold(kt, bap)
                for kp in range(N_KP):
                    for i in range(2):
                        nc.scalar.activation(
                            wp8[kp][:, i, :], wf[2 * kp + i][:],
                            COPY, scale=WS8,
                        )
                fetch_x(0)
                for mt in HEAD_MTS:
                    for oc in range(2):
                        for kp in range(N_KP):
                            dr_pass(mt, oc, hpm[(mt, oc)], kp,
                                    start=(kp == 0))

                # f16 k-range: stream W + fold TWO kt ahead of the k-outer
                # absorption so head passes never sit between BA and the
                # next fold in the Tensor FIFO, nor wait a just-landed fold
                for kt in range(KT16_0, KT):
                    w_dma(kt)
                    ba_fold(kt, bap)
                    if kt == 26:
                        fetch_x(1)
                    prev = kt - 2
                    if KT16_0 <= prev < KT - 1:
                        for mt in HEAD_MTS:
                            for oc in range(2):
                                f16_pass(mt, oc, hpm[(mt, oc)], prev)
                for mt in HEAD_MTS:
                    for oc in range(2):
                        f16_pass(mt, oc, hpm[(mt, oc)], KT - 2)
                fetch_x(2)
                osbh = {}
                for mt in HEAD_MTS:
                    osbh[mt] = outp.tile([128, O_SH], f32, name="osb")
                    for oc in range(2):
                        close_group(mt, oc, hpm[(mt, oc)], osbh[mt])
                    nc.sync.dma_start(
                        out_d[mt * 128 : (mt + 1) * 128, :], osbh[mt][:]
                    )

            # ---- steady phase ----
            with tc.tile_pool(name="sps", bufs=4, space="PSUM") as sps:
                for mt in range(2, N_MT):
                    pr = mt // 2
                    if mt % 2 == 0 and pr + 2 < N_PAIR:
                        fetch_x(pr + 2)
                    osb = outp.tile([128, O_SH], f32, name="osb")
                    last = mt == N_MT - 1
                    for oc in range(2):
                        pm = sps.tile([128, 512], f32, name="pm")
                        for kp in range(N_KP):
                            dr_pass(mt, oc, pm, kp, start=(kp == 0))
                        for kt in range(KT16_0, KT - 1):
                            f16_pass(mt, oc, pm, kt)
                        close_group(mt, oc, pm, osb)
                        if last:
                            hs = slice(oc * 512, (oc + 1) * 512)
                            nc.sync.dma_start(
                                out_d[mt * 128 : (mt + 1) * 128, hs],
                                osb[:, hs],
                            )
                    if not last:
                        nc.sync.dma_start(
                            out_d[mt * 128 : (mt + 1) * 128, :], osb[:]
                        )

    nc.compile()
    return nc


def _get_nc():
    if "nc" not in _NC_CACHE:
        _NC_CACHE["nc"] = _build()
    return _NC_CACHE["nc"]


def kernel(x, weight, bias, A, B):
    global LAST_RESULT
    from concourse.bass_utils import run_bass_kernel_spmd

    x = np.asarray(x, dtype=np.float32).reshape(M, K)
    weight = np.asarray(weight, dtype=np.float32)
    bias = np.asarray(bias, dtype=np.float32)
    A = np.asarray(A, dtype=np.float32)
    B = np.asarray(B, dtype=np.float32)

    import ml_dtypes

    # Host-side layout prep (transposes, zero-pad + dtype casts only).
    xtl_slabs, xq_slabs = [], []
    for mi in range(M_SPLIT):
        xt = x[mi * M_SH : (mi + 1) * M_SH].T  # [K, M_SH]
        v = xt.reshape(KT, 128, N_PAIR, 256).transpose(2, 1, 0, 3)
        v16 = v.astype(np.float16)
        xtl_slabs.append(np.ascontiguousarray(v16[:, :, KT16_0:, :]))
        vq = (v16[:, :, :KT16_0, :].astype(np.float32) / WS8).astype(
            ml_dtypes.float8_e4m3fn
        )
        xq_slabs.append(
            np.ascontiguousarray(vq.reshape(N_PAIR, 128, N_KP, 2, 256))
        )
    a_pad = np.zeros((128, K), dtype=np.float16)
    a_pad[:R] = A.astype(np.float16)
    wtl_slabs, bt_slabs, bias_slabs = [], [], []
    for oi in range(O_SPLIT):
        os_ = slice(oi * O_SH, (oi + 1) * O_SH)
        wt = weight[os_].T  # [K, O_SH]
        wtl_slabs.append(
            np.ascontiguousarray(
                wt.reshape(KT, 128, O_SH), dtype=np.float16
            )
        )
        btp = np.zeros((128, O_SH), dtype=np.float16)
        btp[:R] = (SCALING * B[os_].T).astype(np.float16)
        bt_slabs.append(btp)
        bias_slabs.append(np.ascontiguousarray(bias[os_]))

    nc = _get_nc()
    in_maps = []
    for c in range(N_CORES):
        mi, oi = divmod(c, O_SPLIT)
        in_maps.append(
            {
                "xtl": xtl_slabs[mi],
                "xq": xq_slabs[mi],
                "wtl": wtl_slabs[oi],
                "a": a_pad,
                "bt": bt_slabs[oi],
                "bias": bias_slabs[oi],
            }
        )

    res = run_bass_kernel_spmd(nc, in_maps, list(range(N_CORES)))
    LAST_RESULT = res

    out = np.empty((M, OUT_F), np.float32)
    for c in range(N_CORES):
        mi, oi = divmod(c, O_SPLIT)
        out[mi * M_SH : (mi + 1) * M_SH, oi * O_SH : (oi + 1) * O_SH] = (
            res.results[c]["out"]
        )
    return out.reshape(4, 2048, OUT_F)


# revision 26
# speedup vs baseline: 1.0027x; 1.0027x over previous
"""LoRALinear Trainium2 kernel.

out = x @ W^T + bias + 2.0 * ((x @ A^T) @ B^T)

Strategy (v15):
  - 2x4 sharding over 8 NeuronCores: M split 2-way, out_features 4-way.
    Per core: x-shard [4096 tok, 4096 k], W-shard [1024 out, 4096 k].
  - Host ships k-major pre-tiled f16 layouts (layout + dtype cast only):
    x as [16 pair, 128 p, 32 kt, 256 m] f16 (contiguous 2MiB per pair
    fetch); W as [32 kt, 128 p, 1024 o] f16, DMA'd per k-tile straight
    into its wf tile; A and B^T zero-padded to 128 partitions so their
    DMAs run at line rate (16-partition DMAs only use 2 of 16 ports and
    landed ~14us, stalling the whole BA chain in v6/v7).
  - LoRA fold: rank-16 BA matmul lands in PSUM, then ONE in-place DVE
    add per 512-col half: wf[kt] += (2*B@A)^T[kt]. W_eff production is
    DVE-paced at ~1.2us/kt and overlaps the W/x DMA window (~40us).
  - Mixed precision: k-tiles 0..7 as fp8e4 DoubleRow pairs (4 DR passes
    of FD=512 per group, each covering 2 k-tiles), k-tiles 8..31 f16
    (24 passes of FD=512). fp8 operands scaled W*8 / x*(1/8) (exact
    pow2). ACT-engine f16->f8 conversions are bit-exact RNE
    (probe-verified). Measured rel_max vs cpu reference: 1.87e-2
    (gate 2e-2).
  - Group order: DR kp0 (start=True, full-bank FD=512) .. kp3, then f16
    kt8..kt30, kt31 carries stop; DVE adds bias at eviction.
  - Head: m-tiles 0,1 (x pair 0) x 2 oc absorb the W stream k-outer,
    lagging the fold chain by two k-tiles so absorption passes neither
    join the fold critical path in the Tensor FIFO nor wait on a
    just-landed fold. x pair 1 is fetched late (kt26) to keep the W
    stream at full DMA rate. Steady phase is k-inner per m-tile; final
    m-tile stores per-oc halves.
"""

import numpy as np

IN_F = 4096
OUT_F = 4096
R = 16
SCALING = 2.0
M = 4 * 2048  # 8192 tokens

N_CORES = 8
M_SPLIT = 2
O_SPLIT = 4
M_SH = M // M_SPLIT      # 4096 rows per core
O_SH = OUT_F // O_SPLIT  # 1024 out-features per core
K = IN_F
KT = K // 128            # 32 k-tiles
N_KP = 4                 # fp8 k-pairs = k-tiles 0..7
KT16_0 = 2 * N_KP        # first f16 k-tile (8)
N_MT = M_SH // 128       # 32 m-tiles
N_PAIR = N_MT // 2       # 16 m-tile pairs (x fetched per pair)
WS8 = 8.0                # fp8 scaling: W*8, x/8

_NC_CACHE = {}
LAST_RESULT = None


def _build():
    import concourse.mybir as mybir
    import concourse.tile as tile
    from concourse import bacc

    f32, f16, f8 = mybir.dt.float32, mybir.dt.float16, mybir.dt.float8e4
    DR = mybir.MatmulPerfMode.DoubleRow
    COPY = mybir.ActivationFunctionType.Copy

    nc = bacc.Bacc(
        "TRN2", target_bir_lowering=False, debug=False, num_devices=N_CORES
    )
    xtl_d = nc.dram_tensor("xtl", [N_PAIR, 128, KT, 256], f16,
                           kind="ExternalInput")
    wtl_d = nc.dram_tensor("wtl", [KT, 128, O_SH], f16, kind="ExternalInput")
    a_d = nc.dram_tensor("a", [128, K], f16, kind="ExternalInput")
    bt_d = nc.dram_tensor("bt", [128, O_SH], f16, kind="ExternalInput")
    bias_d = nc.dram_tensor("bias", [O_SH], f32, kind="ExternalInput")
    out_d = nc.dram_tensor("out", [M_SH, O_SH], f32, kind="ExternalOutput")

    with tile.TileContext(nc) as tc:
        with (
            tc.tile_pool(name="const", bufs=1) as const,
            tc.tile_pool(name="wfp", bufs=1) as wfp,
            tc.tile_pool(name="wp8p", bufs=1) as wp8p,
            tc.tile_pool(name="xfp", bufs=3) as xfp,
            tc.tile_pool(name="xpp", bufs=3) as xpp,
            tc.tile_pool(name="outp", bufs=3) as outp,
        ):
            # ---- constants ----
            junk = const.tile([128, 512], f16)
            nc.vector.memset(junk[:], 0.0)
            # A / B^T are zero-padded to 128 partitions for line-rate DMA
            a_sb = const.tile([128, K], f16)
            nc.sync.dma_start(a_sb[:], a_d[:])
            bt2 = const.tile([128, O_SH], f16)  # host ships 2*B^T (exact
            nc.sync.dma_start(bt2[:], bt_d[:])   # pow2 scale, bit-lossless)
            bias_bc = const.tile([128, O_SH], f32)
            nc.gpsimd.dma_start(bias_bc[:], bias_d[:].partition_broadcast(128))

            wf = {kt: wfp.tile([128, O_SH], f16, name=f"wf{kt}")
                  for kt in range(KT)}
            wp8 = [wp8p.tile([128, 2, O_SH], f8, name=f"wp8_{kp}")
                   for kp in range(N_KP)]

            def w_dma(kt):
                nc.sync.dma_start(wf[kt][:], wtl_d[kt, :, :])

            def ba_fold(kt, pba_pool):
                """BA matmul; fold in-place: wf[kt] += (2*B@A)^T[kt]."""
                ks = slice(kt * 128, (kt + 1) * 128)
                for h in range(2):
                    hs = slice(h * 512, (h + 1) * 512)
                    pba = pba_pool.tile([128, 512], f32, name=f"pba{h}")
                    nc.tensor.matmul(
                        pba[:], a_sb[0:R, ks], bt2[0:R, hs],
                        start=True, stop=True,
                    )
                    nc.vector.tensor_add(wf[kt][:, hs], pba[:], wf[kt][:, hs])

            xfs = [None] * N_PAIR  # f16 x tiles [128, KT, 256] per m-pair
            xps = [None] * N_PAIR  # fp8 paired x tiles [128, N_KP, 2, 256]

            def fetch_x(pr):
                xf = xfp.tile([128, KT, 256], f16, name="xf")
                nc.sync.dma_start(xf[:], xtl_d[pr, :, :, :])
                xp = xpp.tile([128, N_KP, 2, 256], f8, name="xp")
                for kp in range(N_KP):
                    for i in range(2):
                        nc.scalar.activation(
                            xp[:, kp, i, :], xf[:, 2 * kp + i, :],
                            COPY, scale=1.0 / WS8,
                        )
                xfs[pr], xps[pr] = xf, xp

            def xslice(mt, kt):
                pr, half = divmod(mt, 2)
                return xfs[pr][:, kt, half * 128 : half * 128 + 128]

            def xslice8(mt, kp):
                pr, half = divmod(mt, 2)
                return xps[pr][:, kp, :, half * 128 : half * 128 + 128]

            def dr_pass(mt, oc, pm, kp, start=False):
                ocs = slice(oc * 512, (oc + 1) * 512)
                nc.tensor.matmul(
                    pm[:], xslice8(mt, kp), wp8[kp][:, :, ocs],
                    start=start, stop=False, perf_mode=DR,
                )

            def f16_pass(mt, oc, pm, kt, start=False):
                nc.tensor.matmul(
                    pm[:], xslice(mt, kt), wf[kt][:, oc * 512 : (oc + 1) * 512],
                    start=start, stop=False,
                )

            def close_group(mt, oc, pm, osb):
                nc.tensor.matmul(
                    pm[:], xslice(mt, KT - 1),
                    wf[KT - 1][:, oc * 512 : (oc + 1) * 512],
                    start=False, stop=True,
                )
                hs = slice(oc * 512, (oc + 1) * 512)
                nc.vector.tensor_add(osb[:, hs], pm[:], bias_bc[:, hs])

            HEAD_MTS = (0, 1)

            with tc.tile_pool(name="bap", bufs=2, space="PSUM") as bap, \
                 tc.tile_pool(name="hps", bufs=1, space="PSUM") as hps:
                # warmup burst flips the PE clock gate early and bridges the
                # gap until the BA stream's inputs land (~9us)
                pwarm = bap.tile([128, 512], f32, name="pba0")
                for _ in range(24):
                    nc.tensor.matmul(
                        pwarm[:], junk[:, 0:128], junk[:], start=True, stop=True
                    )

                hpm = {
                    (mt, oc): hps.tile([128, 512], f32, name=f"h{mt}_{oc}")
                    for mt in HEAD_MTS for oc in range(2)
                }

                # fp8 k-range first: W + BA folds for kt 0..7, quantize, then
                # open the head groups with a dense DR burst (re-arms HAM)
                for kt in range(KT16_0):
                    w_dma(kt)
                    ba_fold(kt, bap)
                for kp in range(N_KP):
                    for i in range(2):
                        nc.scalar.activation(
                            wp8[kp][:, i, :], wf[2 * kp + i][:],
                            COPY, scale=WS8,
                        )
                fetch_x(0)
                for mt in HEAD_MTS:
                    for oc in range(2):
                        for kp in range(N_KP):
                            dr_pass(mt, oc, hpm[(mt, oc)], kp,
                                    start=(kp == 0))

                # f16 k-range: stream W + fold TWO kt ahead of the k-outer
                # absorption so head passes never sit between BA and the
                # next fold in the Tensor FIFO, nor wait a just-landed fold
                for kt in range(KT16_0, KT):
                    w_dma(kt)
                    ba_fold(kt, bap)
                    if kt == 26:
                        fetch_x(1)
                    prev = kt - 3
                    if KT16_0 <= prev < KT - 1:
                        for mt in HEAD_MTS:
                            for oc in range(2):
                                f16_pass(mt, oc, hpm[(mt, oc)], prev)
                for tail_kt in (KT - 3, KT - 2):
                    for mt in HEAD_MTS:
                        for oc in range(2):
                            f16_pass(mt, oc, hpm[(mt, oc)], tail_kt)
                fetch_x(2)
                osbh = {}
                for mt in HEAD_MTS:
                    osbh[mt] = outp.tile([128, O_SH], f32, name="osb")
                    for oc in range(2):
                        close_group(mt, oc, hpm[(mt, oc)], osbh[mt])
                    nc.sync.dma_start(
                        out_d[mt * 128 : (mt + 1) * 128, :], osbh[mt][:]
                    )

            # ---- steady phase ----
            with tc.tile_pool(name="sps", bufs=4, space="PSUM") as sps:
                for mt in range(2, N_MT):
                    pr = mt // 2
                    if mt % 2 == 0 and pr + 2 < N_PAIR:
                        fetch_x(pr + 2)
                    osb = outp.tile([128, O_SH], f32, name="osb")
                    last = mt == N_MT - 1
                    for oc in range(2):
                        pm = sps.tile([128, 512], f32, name="pm")
                        # DR passes spread among f16 passes: back-to-back DR
                        # saturates the LDW path (213ns ~= the 215.8ns MM);
                        # interleaved, each DR LDW hides under an f16 MM
                        dr_pass(mt, oc, pm, 0, start=True)
                        for kt in range(KT16_0, KT - 1):
                            f16_pass(mt, oc, pm, kt)
                            if kt in (13, 19, 25):
                                dr_pass(mt, oc, pm, (kt - 7) // 6)
                        close_group(mt, oc, pm, osb)
                        if last:
                            hs = slice(oc * 512, (oc + 1) * 512)
                            nc.sync.dma_start(
                                out_d[mt * 128 : (mt + 1) * 128, hs],
                                osb[:, hs],
                            )
                    if not last:
                        nc.sync.dma_start(
                            out_d[mt * 128 : (mt + 1) * 128, :], osb[:]
                        )

    nc.compile()
    return nc


def _get_nc():
    if "nc" not in _NC_CACHE:
        _NC_CACHE["nc"] = _build()
    return _NC_CACHE["nc"]


def kernel(x, weight, bias, A, B):
    global LAST_RESULT
    from concourse.bass_utils import run_bass_kernel_spmd

    x = np.asarray(x, dtype=np.float32).reshape(M, K)
    weight = np.asarray(weight, dtype=np.float32)
    bias = np.asarray(bias, dtype=np.float32)
    A = np.asarray(A, dtype=np.float32)
    B = np.asarray(B, dtype=np.float32)

    # Host-side layout prep (transposes, zero-pad + f16 casts only).
    xtl_slabs = []
    for mi in range(M_SPLIT):
        xt = x[mi * M_SH : (mi + 1) * M_SH].T  # [K, M_SH]
        v = xt.reshape(KT, 128, N_PAIR, 256).transpose(2, 1, 0, 3)
        xtl_slabs.append(np.ascontiguousarray(v, dtype=np.float16))
    a_pad = np.zeros((128, K), dtype=np.float16)
    a_pad[:R] = A.astype(np.float16)
    wtl_slabs, bt_slabs, bias_slabs = [], [], []
    for oi in range(O_SPLIT):
        os_ = slice(oi * O_SH, (oi + 1) * O_SH)
        wt = weight[os_].T  # [K, O_SH]
        wtl_slabs.append(
            np.ascontiguousarray(
                wt.reshape(KT, 128, O_SH), dtype=np.float16
            )
        )
        btp = np.zeros((128, O_SH), dtype=np.float16)
        btp[:R] = (SCALING * B[os_].T).astype(np.float16)
        bt_slabs.append(btp)
        bias_slabs.append(np.ascontiguousarray(bias[os_]))

    nc = _get_nc()
    in_maps = []
    for c in range(N_CORES):
        mi, oi = divmod(c, O_SPLIT)
        in_maps.append(
            {
                "xtl": xtl_slabs[mi],
                "wtl": wtl_slabs[oi],
                "a": a_pad,
                "bt": bt_slabs[oi],
                "bias": bias_slabs[oi],
            }
        )

    res = run_bass_kernel_spmd(nc, in_maps, list(range(N_CORES)))
    LAST_RESULT = res

    out = np.empty((M, OUT_F), np.float32)
    for c in range(N_CORES):
        mi, oi = divmod(c, O_SPLIT)
        out[mi * M_SH : (mi + 1) * M_SH, oi * O_SH : (oi + 1) * O_SH] = (
            res.results[c]["out"]
        )
    return out.reshape(4, 2048, OUT_F)


# revision 28
# speedup vs baseline: 1.0041x; 1.0015x over previous
"""LoRALinear Trainium2 kernel.

out = x @ W^T + bias + 2.0 * ((x @ A^T) @ B^T)

Strategy (v15):
  - 2x4 sharding over 8 NeuronCores: M split 2-way, out_features 4-way.
    Per core: x-shard [4096 tok, 4096 k], W-shard [1024 out, 4096 k].
  - Host ships k-major pre-tiled f16 layouts (layout + dtype cast only):
    x as [16 pair, 128 p, 32 kt, 256 m] f16 (contiguous 2MiB per pair
    fetch); W as [32 kt, 128 p, 1024 o] f16, DMA'd per k-tile straight
    into its wf tile; A and B^T zero-padded to 128 partitions so their
    DMAs run at line rate (16-partition DMAs only use 2 of 16 ports and
    landed ~14us, stalling the whole BA chain in v6/v7).
  - LoRA fold: rank-16 BA matmul lands in PSUM, then ONE in-place DVE
    add per 512-col half: wf[kt] += (2*B@A)^T[kt]. W_eff production is
    DVE-paced at ~1.2us/kt and overlaps the W/x DMA window (~40us).
  - Mixed precision: k-tiles 0..7 as fp8e4 DoubleRow pairs (4 DR passes
    of FD=512 per group, each covering 2 k-tiles), k-tiles 8..31 f16
    (24 passes of FD=512). fp8 operands scaled W*8 / x*(1/8) (exact
    pow2). ACT-engine f16->f8 conversions are bit-exact RNE
    (probe-verified). Measured rel_max vs cpu reference: 1.87e-2
    (gate 2e-2).
  - Group order: DR kp0 (start=True, full-bank FD=512) .. kp3, then f16
    kt8..kt30, kt31 carries stop; DVE adds bias at eviction.
  - Head: m-tiles 0,1 (x pair 0) x 2 oc absorb the W stream k-outer,
    lagging the fold chain by two k-tiles so absorption passes neither
    join the fold critical path in the Tensor FIFO nor wait on a
    just-landed fold. x pair 1 is fetched late (kt26) to keep the W
    stream at full DMA rate. Steady phase is k-inner per m-tile; final
    m-tile stores per-oc halves.
"""

import numpy as np

IN_F = 4096
OUT_F = 4096
R = 16
SCALING = 2.0
M = 4 * 2048  # 8192 tokens

N_CORES = 8
M_SPLIT = 2
O_SPLIT = 4
M_SH = M // M_SPLIT      # 4096 rows per core
O_SH = OUT_F // O_SPLIT  # 1024 out-features per core
K = IN_F
KT = K // 128            # 32 k-tiles
N_KP = 4                 # fp8 k-pairs = k-tiles 0..7
KT16_0 = 2 * N_KP        # first f16 k-tile (8)
N_MT = M_SH // 128       # 32 m-tiles
N_PAIR = N_MT // 2       # 16 m-tile pairs (x fetched per pair)
WS8 = 8.0                # fp8 scaling: W*8, x/8

_NC_CACHE = {}
LAST_RESULT = None


def _build():
    import concourse.mybir as mybir
    import concourse.tile as tile
    from concourse import bacc

    f32, f16, f8 = mybir.dt.float32, mybir.dt.float16, mybir.dt.float8e4
    DR = mybir.MatmulPerfMode.DoubleRow
    COPY = mybir.ActivationFunctionType.Copy

    nc = bacc.Bacc(
        "TRN2", target_bir_lowering=False, debug=False, num_devices=N_CORES
    )
    xtl_d = nc.dram_tensor("xtl", [N_PAIR, 128, KT, 256], f16,
                           kind="ExternalInput")
    wtl_d = nc.dram_tensor("wtl", [KT, 128, O_SH], f16, kind="ExternalInput")
    a_d = nc.dram_tensor("a", [128, K], f16, kind="ExternalInput")
    bt_d = nc.dram_tensor("bt", [128, O_SH], f16, kind="ExternalInput")
    bias_d = nc.dram_tensor("bias", [O_SH], f32, kind="ExternalInput")
    out_d = nc.dram_tensor("out", [M_SH, O_SH], f32, kind="ExternalOutput")

    with tile.TileContext(nc) as tc:
        with (
            tc.tile_pool(name="const", bufs=1) as const,
            tc.tile_pool(name="wfp", bufs=1) as wfp,
            tc.tile_pool(name="wp8p", bufs=1) as wp8p,
            tc.tile_pool(name="xfp", bufs=3) as xfp,
            tc.tile_pool(name="xpp", bufs=3) as xpp,
            tc.tile_pool(name="outp", bufs=3) as outp,
        ):
            # ---- constants ----
            junk = const.tile([128, 512], f16)
            nc.vector.memset(junk[:], 0.0)
            # A / B^T are zero-padded to 128 partitions for line-rate DMA
            a_sb = const.tile([128, K], f16)
            nc.sync.dma_start(a_sb[:], a_d[:])
            bt2 = const.tile([128, O_SH], f16)  # host ships 2*B^T (exact
            nc.sync.dma_start(bt2[:], bt_d[:])   # pow2 scale, bit-lossless)
            bias_bc = const.tile([128, O_SH], f32)
            nc.gpsimd.dma_start(bias_bc[:], bias_d[:].partition_broadcast(128))

            wf = {kt: wfp.tile([128, O_SH], f16, name=f"wf{kt}")
                  for kt in range(KT)}
            wp8 = [wp8p.tile([128, 2, O_SH], f8, name=f"wp8_{kp}")
                   for kp in range(N_KP)]

            def w_dma(kt):
                nc.sync.dma_start(wf[kt][:], wtl_d[kt, :, :])

            def ba_fold(kt, pba_pool):
                """BA matmul; fold in-place: wf[kt] += (2*B@A)^T[kt]."""
                ks = slice(kt * 128, (kt + 1) * 128)
                for h in range(2):
                    hs = slice(h * 512, (h + 1) * 512)
                    pba = pba_pool.tile([128, 512], f32, name=f"pba{h}")
                    nc.tensor.matmul(
                        pba[:], a_sb[0:R, ks], bt2[0:R, hs],
                        start=True, stop=True,
                    )
                    nc.vector.tensor_add(wf[kt][:, hs], pba[:], wf[kt][:, hs])

            xfs = [None] * N_PAIR  # f16 x tiles [128, KT, 256] per m-pair
            xps = [None] * N_PAIR  # fp8 paired x tiles [128, N_KP, 2, 256]

            def fetch_x(pr):
                xf = xfp.tile([128, KT, 256], f16, name="xf")
                nc.sync.dma_start(xf[:], xtl_d[pr, :, :, :])
                xp = xpp.tile([128, N_KP, 2, 256], f8, name="xp")
                for kp in range(N_KP):
                    for i in range(2):
                        nc.scalar.activation(
                            xp[:, kp, i, :], xf[:, 2 * kp + i, :],
                            COPY, scale=1.0 / WS8,
                        )
                xfs[pr], xps[pr] = xf, xp

            def xslice(mt, kt):
                pr, half = divmod(mt, 2)
                return xfs[pr][:, kt, half * 128 : half * 128 + 128]

            def xslice8(mt, kp):
                pr, half = divmod(mt, 2)
                return xps[pr][:, kp, :, half * 128 : half * 128 + 128]

            def dr_pass(mt, oc, pm, kp, start=False):
                ocs = slice(oc * 512, (oc + 1) * 512)
                nc.tensor.matmul(
                    pm[:], xslice8(mt, kp), wp8[kp][:, :, ocs],
                    start=start, stop=False, perf_mode=DR,
                )

            def f16_pass(mt, oc, pm, kt, start=False):
                nc.tensor.matmul(
                    pm[:], xslice(mt, kt), wf[kt][:, oc * 512 : (oc + 1) * 512],
                    start=start, stop=False,
                )

            def close_group(mt, oc, pm, osb):
                nc.tensor.matmul(
                    pm[:], xslice(mt, KT - 1),
                    wf[KT - 1][:, oc * 512 : (oc + 1) * 512],
                    start=False, stop=True,
                )
                hs = slice(oc * 512, (oc + 1) * 512)
                nc.vector.tensor_add(osb[:, hs], pm[:], bias_bc[:, hs])

            HEAD_MTS = (0, 1)

            with tc.tile_pool(name="bap", bufs=2, space="PSUM") as bap, \
                 tc.tile_pool(name="hps", bufs=1, space="PSUM") as hps:
                # warmup burst flips the PE clock gate early and bridges the
                # gap until the BA stream's inputs land (~9us)
                pwarm = bap.tile([128, 512], f32, name="pba0")
                for _ in range(18):
                    nc.tensor.matmul(
                        pwarm[:], junk[:, 0:128], junk[:], start=True, stop=True
                    )

                hpm = {
                    (mt, oc): hps.tile([128, 512], f32, name=f"h{mt}_{oc}")
                    for mt in HEAD_MTS for oc in range(2)
                }

                # fp8 k-range first: W + BA folds for kt 0..7, quantize, then
                # open the head groups with a dense DR burst (re-arms HAM)
                for kt in range(KT16_0):
                    w_dma(kt)
                    ba_fold(kt, bap)
                for kp in range(N_KP):
                    for i in range(2):
                        nc.scalar.activation(
                            wp8[kp][:, i, :], wf[2 * kp + i][:],
                            COPY, scale=WS8,
                        )
                fetch_x(0)
                for mt in HEAD_MTS:
                    for oc in range(2):
                        for kp in range(N_KP):
                            dr_pass(mt, oc, hpm[(mt, oc)], kp,
                                    start=(kp == 0))

                # f16 k-range: stream W + fold TWO kt ahead of the k-outer
                # absorption so head passes never sit between BA and the
                # next fold in the Tensor FIFO, nor wait a just-landed fold
                for kt in range(KT16_0, KT):
                    w_dma(kt)
                    ba_fold(kt, bap)
                    if kt == 26:
                        fetch_x(1)
                    prev = kt - 2
                    if KT16_0 <= prev < KT - 1:
                        for mt in HEAD_MTS:
                            for oc in range(2):
                                f16_pass(mt, oc, hpm[(mt, oc)], prev)
                for mt in HEAD_MTS:
                    for oc in range(2):
                        f16_pass(mt, oc, hpm[(mt, oc)], KT - 2)
                fetch_x(2)
                osbh = {}
                for mt in HEAD_MTS:
                    osbh[mt] = outp.tile([128, O_SH], f32, name="osb")
                    for oc in range(2):
                        close_group(mt, oc, hpm[(mt, oc)], osbh[mt])
                    nc.sync.dma_start(
                        out_d[mt * 128 : (mt + 1) * 128, :], osbh[mt][:]
                    )

            # ---- steady phase ----
            with tc.tile_pool(name="sps", bufs=4, space="PSUM") as sps:
                for mt in range(2, N_MT):
                    pr = mt // 2
                    if mt % 2 == 0 and pr + 2 < N_PAIR:
                        fetch_x(pr + 2)
                    osb = outp.tile([128, O_SH], f32, name="osb")
                    last = mt == N_MT - 1
                    for oc in range(2):
                        pm = sps.tile([128, 512], f32, name="pm")
                        for kp in range(N_KP):
                            dr_pass(mt, oc, pm, kp, start=(kp == 0))
                        for kt in range(KT16_0, KT - 1):
                            f16_pass(mt, oc, pm, kt)
                        close_group(mt, oc, pm, osb)
                        if last:
                            hs = slice(oc * 512, (oc + 1) * 512)
                            nc.sync.dma_start(
                                out_d[mt * 128 : (mt + 1) * 128, hs],
                                osb[:, hs],
                            )
                    if not last:
                        nc.sync.dma_start(
                            out_d[mt * 128 : (mt + 1) * 128, :], osb[:]
                        )

    nc.compile()
    return nc


def _get_nc():
    if "nc" not in _NC_CACHE:
        _NC_CACHE["nc"] = _build()
    return _NC_CACHE["nc"]


def kernel(x, weight, bias, A, B):
    global LAST_RESULT
    from concourse.bass_utils import run_bass_kernel_spmd

    x = np.asarray(x, dtype=np.float32).reshape(M, K)
    weight = np.asarray(weight, dtype=np.float32)
    bias = np.asarray(bias, dtype=np.float32)
    A = np.asarray(A, dtype=np.float32)
    B = np.asarray(B, dtype=np.float32)

    # Host-side layout prep (transposes, zero-pad + f16 casts only).
    xtl_slabs = []
    for mi in range(M_SPLIT):
        xt = x[mi * M_SH : (mi + 1) * M_SH].T  # [K, M_SH]
        v = xt.reshape(KT, 128, N_PAIR, 256).transpose(2, 1, 0, 3)
        xtl_slabs.append(np.ascontiguousarray(v, dtype=np.float16))
    a_pad = np.zeros((128, K), dtype=np.float16)
    a_pad[:R] = A.astype(np.float16)
    wtl_slabs, bt_slabs, bias_slabs = [], [], []
    for oi in range(O_SPLIT):
        os_ = slice(oi * O_SH, (oi + 1) * O_SH)
        wt = weight[os_].T  # [K, O_SH]
        wtl_slabs.append(
            np.ascontiguousarray(
                wt.reshape(KT, 128, O_SH), dtype=np.float16
            )
        )
        btp = np.zeros((128, O_SH), dtype=np.float16)
        btp[:R] = (SCALING * B[os_].T).astype(np.float16)
        bt_slabs.append(btp)
        bias_slabs.append(np.ascontiguousarray(bias[os_]))

    nc = _get_nc()
    in_maps = []
    for c in range(N_CORES):
        mi, oi = divmod(c, O_SPLIT)
        in_maps.append(
            {
                "xtl": xtl_slabs[mi],
                "wtl": wtl_slabs[oi],
                "a": a_pad,
                "bt": bt_slabs[oi],
                "bias": bias_slabs[oi],
            }
        )

    res = run_bass_kernel_spmd(nc, in_maps, list(range(N_CORES)))
    LAST_RESULT = res

    out = np.empty((M, OUT_F), np.float32)
    for c in range(N_CORES):
        mi, oi = divmod(c, O_SPLIT)
        out[mi * M_SH : (mi + 1) * M_SH, oi * O_SH : (oi + 1) * O_SH] = (
            res.results[c]["out"]
        )
    return out.reshape(4, 2048, OUT_F)
